# revision 1
# baseline (speedup 1.0000x reference)
"""Trainium2 Bass kernel for nn_Decoder (LSTM decoder + big output projection).

Model (VOCAB=32000, EM=256, UNITS=512, B=64, T=16):
  em     = emb_table[inputs]                      # [B,T,EM]
  xz     = em @ Wx + b                            # [B,T,4U] (precomputed input gates)
  scan:    z = xz_t + h @ Wh ; i,f,g,o = sigmoid(z)
           c = f*c + i*g ; h = o*sigmoid(c)       # 16 sequential steps
  logits = concat_t(h_t) @ Wout + bout            # [B, 8192] @ [8192, 32000]
  out    = softmax(logits)

Distribution over 8 NeuronCores:
  - The scan is replicated on every core (it is tiny and serial; replicating it
    avoids an AllGather of the hidden states).
  - Wout / bout / softmax are column-sharded: core c owns vocab columns
    [c*4000, (c+1)*4000).  Only the softmax denominator (a [64,1] row-sum)
    crosses cores, via one AllReduce.
  - Weights are shipped to the device as bf16 (the 1 GB Wout read is the
    roofline floor; bf16 halves it).  All accumulation is fp32 in PSUM.

On-chip layout is "transposed": hidden state and gates live as [unit, batch]
tiles ([128 partitions, 64 batch]) so the recurrent matmul uses Wh as the
stationary operand and no per-step transposes are needed.  The per-step
output-projection partials (stationary = h_t^T, moving = streamed Wout rows)
run while the next step's gate math is on the Vector/Scalar engines, and the
Wout DMA stream overlaps everything.
"""

import numpy as np
import ml_dtypes
from contextlib import ExitStack

import concourse.bacc as bacc
import concourse.mybir as mybir
import concourse.tile as tile
from concourse.bass_utils import run_bass_kernel_spmd

VOCAB, EM, UNITS, B, T = 32000, 256, 512, 64, 16
NCORES = 8
VS = VOCAB // NCORES          # 4000 vocab columns per core
GU = 4 * UNITS                # 2048 gate units
NJ = GU // 128                # 16 gate m-tiles
KH = UNITS // 128             # 4 k-tiles of the hidden state
KE = EM // 128                # 2 k-tiles of the embedding
NTOK = B * T                  # 1024 tokens
NCH = 8                       # output-projection n-chunks per core
CH = VS // NCH                # 500 columns per chunk (<=512 PSUM bank limit)

BF16 = mybir.dt.bfloat16
F32 = mybir.dt.float32

_prog_cache = {}


def _build_program(_collective=True, _compile=True):
    """Trace + compile the single-core SPMD program (cached per process).

    _collective=False swaps the AllReduce for a local copy (used only for
    single-core cost-model profiling, never for real runs)."""
    key = ("nc", _collective)
    if key in _prog_cache:
        return _prog_cache[key]

    nc = bacc.Bacc("TRN2", target_bir_lowering=False, debug=False,
                   num_devices=NCORES if _collective else 1)

    emt_d = nc.dram_tensor("emt", [128, KE, NTOK], BF16, kind="ExternalInput").ap()
    wx_d = nc.dram_tensor("wx", [128, KE, GU], BF16, kind="ExternalInput").ap()
    wh_d = nc.dram_tensor("wh", [128, KH, GU], BF16, kind="ExternalInput").ap()
    bt_d = nc.dram_tensor("bt", [128, NJ], F32, kind="ExternalInput").ap()
    h0_d = nc.dram_tensor("h0", [128, KH, B], BF16, kind="ExternalInput").ap()
    id_d = nc.dram_tensor("ident", [128, 128], BF16, kind="ExternalInput").ap()
    c0_d = nc.dram_tensor("c0", [128, KH * B], F32, kind="ExternalInput").ap()
    wout_d = nc.dram_tensor("wout", [T * UNITS, VS], BF16,
                            kind="ExternalInput").ap()
    boutb_d = nc.dram_tensor("boutb", [B, VS], F32, kind="ExternalInput").ap()
    out_d = nc.dram_tensor("probs", [B, VS], F32, kind="ExternalOutput").ap()
    cc_in = nc.dram_tensor("cc_in", [B, 1], F32).ap()
    cc_out = nc.dram_tensor("cc_out", [B, 1], F32, addr_space="Shared").ap()

    # [8192, VS] viewed as [128 partitions, 64 k-tiles, VS]
    wout_r = wout_d.rearrange("(s p) n -> p s n", p=128)

    gather_sem = nc.alloc_semaphore("gather_sem")
    cc_sem = nc.alloc_semaphore("cc_sem")
    cc_done_sem = nc.alloc_semaphore("cc_done_sem")

    with tile.TileContext(nc) as tc, ExitStack() as ctx:
        consts = ctx.enter_context(tc.tile_pool(name="consts", bufs=1))
        wout_pool = ctx.enter_context(tc.tile_pool(name="wout", bufs=6))
        psum_big = ctx.enter_context(tc.tile_pool(name="psb", bufs=2, space="PSUM"))
        psum_proj = ctx.enter_context(tc.tile_pool(name="psp", bufs=4, space="PSUM"))
        work = ctx.enter_context(tc.tile_pool(name="work", bufs=1))

        # ---- resident tensors (xz-phase inputs are DMA'd first so the PE can
        # start before the larger Wh / logits loads land) ----------------------
        wx_sb = consts.tile([128, KE, GU], BF16, tag="wx")
        nc.sync.dma_start(out=wx_sb[:], in_=wx_d[:])
        emt = consts.tile([128, KE, NTOK], BF16, tag="emt")
        nc.sync.dma_start(out=emt[:], in_=emt_d[:])
        bt_sb = consts.tile([128, NJ], F32, tag="bt")
        nc.sync.dma_start(out=bt_sb[:], in_=bt_d[:])
        id_sb = consts.tile([128, 128], BF16, tag="ident")
        nc.sync.dma_start(out=id_sb[:], in_=id_d[:])
        wh_sb = consts.tile([128, KH, GU], BF16, tag="wh")
        nc.sync.dma_start(out=wh_sb[:], in_=wh_d[:])
        c_sb = consts.tile([128, KH * B], F32, tag="c")
        nc.sync.dma_start(out=c_sb[:], in_=c0_d[:])
        # hidden states for all steps (slot 0 = initial state), bf16 transposed
        hs_sb = consts.tile([128, T + 1, KH, B], BF16, tag="hs")
        nc.sync.dma_start(out=hs_sb[:, 0, :, :], in_=h0_d[:])
        # logits accumulator, initialised with bout (pre-broadcast by the host)
        logits = consts.tile([B, VS], F32, tag="logits")
        nc.sync.dma_start(out=logits[:], in_=boutb_d[:])
        # xz = em @ Wx + b, transposed layout [gate-unit, (t, b)]
        xz_sb = consts.tile([128, NJ, T, B], BF16, tag="xz")

        # ---- xz = Wx^T @ em^T  (+ b folded in during PSUM evacuation) --------
        for j in range(NJ):
            ps = psum_big.tile([128, T * B], F32, tag="zps")
            for kt in range(KE):
                for nh in range(2):
                    nc.tensor.matmul(
                        ps[:, nh * 512:(nh + 1) * 512],
                        wx_sb[:, kt, j * 128:(j + 1) * 128],
                        emt[:, kt, nh * 512:(nh + 1) * 512],
                        start=(kt == 0), stop=(kt == KE - 1),
                    )
            nc.vector.tensor_scalar_add(
                xz_sb[:, j, :, :].rearrange("p t b -> p (t b)"),
                ps[:], bt_sb[:, j:j + 1])

        # ---- the scan + interleaved output projection ------------------------
        # Emission order matters for the scheduler: within step t we emit
        # z(t) matmuls FIRST, then the projection for step t-1, then the
        # gate math for t.  That way the PE chews on proj(t-1) while the
        # Vector/Scalar engines run gates(t) — without this the PE idles
        # ~5us per step waiting for h(t).
        def emit_proj(t, wts):
            # logits += h_t @ Wout[512t:512(t+1), :]  (t==0 initialises
            # the accumulator with bout broadcast across partitions)
            for j in range(NCH):
                pp = psum_proj.tile([B, CH], F32, tag="pp")
                for kt in range(KH):
                    nc.tensor.matmul(
                        pp[:],
                        hs_sb[:, t + 1, kt, :],
                        wts[kt // 2][:, kt % 2, j * CH:(j + 1) * CH],
                        start=(kt == 0), stop=(kt == KH - 1),
                    )
                nc.vector.tensor_tensor(
                    logits[:, j * CH:(j + 1) * CH], pp[:],
                    logits[:, j * CH:(j + 1) * CH], mybir.AluOpType.add)

        prev_wts = None
        for t in range(T):
            # stream this step's 512 Wout rows (2 halves of 2 k-tiles each)
            wts = []
            for half in range(2):
                wt = wout_pool.tile([128, 2, VS], BF16, tag="wt")
                nc.sync.dma_start(
                    out=wt[:], in_=wout_r[:, 4 * t + 2 * half: 4 * t + 2 * half + 2, :])
                wts.append(wt)

            # z^T = xz_t + Wh^T @ h^T   [2048 units, 64 batch] in PSUM.
            # xz_t enters the accumulation group via an identity matmul so
            # no separate Vector-engine add sits on the serial chain.
            zt = psum_big.tile([128, NJ * B], F32, tag="zps")
            ztv = zt.rearrange("p (j b) -> p j b", b=B)
            for j in range(NJ):
                nc.tensor.matmul(ztv[:, j, :], id_sb[:],
                                 xz_sb[:, j, t, :], start=True, stop=False)
                for kt in range(KH):
                    nc.tensor.matmul(
                        ztv[:, j, :],
                        wh_sb[:, kt, j * 128:(j + 1) * 128],
                        hs_sb[:, t, kt, :],
                        start=False, stop=(kt == KH - 1),
                    )
            # all four gates use sigmoid; host permuted gate columns to
            # [i, g, f, o] so the activation can run in two halves and the
            # i*g product starts while f/o are still on the Scalar engine
            a_sb = work.tile([128, NJ, B], F32, tag="a")
            nc.scalar.activation(a_sb[:, 0:8, :], ztv[:, 0:8, :],
                                 mybir.ActivationFunctionType.Sigmoid)
            nc.scalar.activation(a_sb[:, 8:16, :], ztv[:, 8:16, :],
                                 mybir.ActivationFunctionType.Sigmoid)
            iT = a_sb[:, 0:4, :].rearrange("p j b -> p (j b)")
            gT = a_sb[:, 4:8, :].rearrange("p j b -> p (j b)")
            fT = a_sb[:, 8:12, :].rearrange("p j b -> p (j b)")
            oT = a_sb[:, 12:16, :].rearrange("p j b -> p (j b)")
            t1 = work.tile([128, KH * B], F32, tag="t1")
            t2 = work.tile([128, KH * B], F32, tag="t2")
            nc.vector.tensor_mul(t1[:], iT, gT)
            nc.vector.tensor_mul(t2[:], fT, c_sb[:])
            nc.vector.tensor_add(c_sb[:], t1[:], t2[:])
            sc = work.tile([128, KH * B], F32, tag="sc")
            nc.scalar.activation(sc[:], c_sb[:],
                                 mybir.ActivationFunctionType.Sigmoid)
            nc.vector.tensor_mul(
                hs_sb[:, t + 1, :, :].rearrange("p k b -> p (k b)"), oT, sc[:])
            if prev_wts is not None:
                emit_proj(t - 1, prev_wts)
            prev_wts = wts

        # ---- final projection step, interleaved with the first softmax pass:
        # exp of chunk j (with a per-chunk row-sum accumulator) starts as soon
        # as that chunk's last evacuation lands, hiding exp#1 under proj(15).
        # the exp scratch reuses a Wout-pool slot (same 16000 B size, and the
        # stream is finished by now) so it costs no extra SBUF
        exps = wout_pool.tile([B, VS], F32, tag="wt")
        ssum8 = consts.tile([B, NCH], F32, tag="ssum8")
        t = T - 1
        for j in range(NCH):
            pp = psum_proj.tile([B, CH], F32, tag="pp")
            for kt in range(KH):
                nc.tensor.matmul(
                    pp[:],
                    hs_sb[:, t + 1, kt, :],
                    prev_wts[kt // 2][:, kt % 2, j * CH:(j + 1) * CH],
                    start=(kt == 0), stop=(kt == KH - 1),
                )
            nc.vector.tensor_tensor(
                logits[:, j * CH:(j + 1) * CH], pp[:],
                logits[:, j * CH:(j + 1) * CH], mybir.AluOpType.add)
            nc.scalar.activation(
                exps[:, j * CH:(j + 1) * CH], logits[:, j * CH:(j + 1) * CH],
                mybir.ActivationFunctionType.Exp, accum_out=ssum8[:, j:j + 1])

        # ---- softmax (vocab-sharded; AllReduce the denominator) --------------
        ssum = consts.tile([B, 1], F32, tag="ssum")
        nc.vector.reduce_sum(ssum[:], ssum8[:], axis=mybir.AxisListType.X)
        gsum = consts.tile([B, 1], F32, tag="gsum")
        if _collective:
            with tc.tile_critical():
                nc.gpsimd.dma_start(out=cc_in[:], in_=ssum[:]).then_inc(cc_sem, 16)
                nc.gpsimd.wait_ge(cc_sem, 16)
                nc.gpsimd.collective_compute(
                    "AllReduce", mybir.AluOpType.add,
                    replica_groups=[list(range(NCORES))],
                    ins=[cc_in[:]], outs=[cc_out[:]],
                ).then_inc(cc_done_sem, 1)
                nc.gpsimd.wait_ge(cc_done_sem, 1)
                nc.gpsimd.dma_start(out=gsum[:], in_=cc_out[:]).then_inc(cc_sem, 16)
                nc.gpsimd.wait_ge(cc_sem, 32)
        else:
            nc.vector.tensor_copy(gsum[:], ssum[:])
        # probs = exp(logits - ln(gsum)) — the bias input of the activation
        # replaces a full-width divide
        lng = consts.tile([B, 1], F32, tag="lng")
        nc.scalar.activation(lng[:], gsum[:], mybir.ActivationFunctionType.Ln)
        negl = consts.tile([B, 1], F32, tag="negl")
        nc.vector.tensor_scalar_mul(negl[:], lng[:], -1.0)
        # second pass chunked so the output DMA streams behind the activation
        for j in range(NCH):
            nc.scalar.activation(
                exps[:, j * CH:(j + 1) * CH], logits[:, j * CH:(j + 1) * CH],
                mybir.ActivationFunctionType.Exp, bias=negl[:, 0:1])
            nc.sync.dma_start(out=out_d[:, j * CH:(j + 1) * CH],
                              in_=exps[:, j * CH:(j + 1) * CH])

    if _compile:
        nc.compile()
    _prog_cache[key] = nc
    return nc


def _prep_in_maps(inputs):
    bf = ml_dtypes.bfloat16
    tok = np.asarray(inputs["inputs"]).astype(np.int64)        # [B, T]
    enc_h = np.asarray(inputs["enc_h"], np.float32)            # [B, U]
    enc_c = np.asarray(inputs["enc_c"], np.float32)            # [B, U]
    emb = np.asarray(inputs["emb_table"], np.float32)          # [V, EM]
    Wx = np.asarray(inputs["Wx"], np.float32)                  # [EM, 4U]
    Wh = np.asarray(inputs["Wh"], np.float32)                  # [U, 4U]
    b = np.asarray(inputs["b"], np.float32)                    # [4U]
    Wout = np.asarray(inputs["Wout"], np.float32)              # [T*U, V]
    bout = np.asarray(inputs["bout"], np.float32)              # [V]

    # embedding lookup on host (pure data movement), shipped pre-transposed:
    # emt[p, k, i] = emb[tok_i, k*128 + p] with token order i = t*B + b
    em_flat = emb[tok.T.reshape(-1)]                           # [NTOK, EM]
    emt = em_flat.reshape(NTOK, KE, 128).transpose(2, 1, 0).astype(bf)

    # permute gate columns i,f,g,o -> i,g,f,o (lets the device split the
    # sigmoid into [i,g] / [f,o] halves)
    perm = np.r_[0:UNITS, 2 * UNITS:3 * UNITS, UNITS:2 * UNITS, 3 * UNITS:GU]
    Wx = Wx[:, perm]
    Wh = Wh[:, perm]
    b = b[perm]

    common = {
        "emt": np.ascontiguousarray(emt),
        "wx": np.ascontiguousarray(
            Wx.reshape(KE, 128, GU).transpose(1, 0, 2).astype(bf)),
        "wh": np.ascontiguousarray(
            Wh.reshape(KH, 128, GU).transpose(1, 0, 2).astype(bf)),
        "bt": np.ascontiguousarray(b.reshape(NJ, 128).T),
        "h0": np.ascontiguousarray(
            enc_h.T.reshape(KH, 128, B).transpose(1, 0, 2).astype(bf)),
        "ident": np.eye(128, dtype=bf),
        "c0": np.ascontiguousarray(
            enc_c.T.reshape(KH, 128, B).transpose(1, 0, 2).reshape(128, KH * B)),
    }
    wout_bf = Wout.astype(bf)
    in_maps = []
    for c in range(NCORES):
        m = dict(common)
        m["wout"] = np.ascontiguousarray(wout_bf[:, c * VS:(c + 1) * VS])
        m["boutb"] = np.ascontiguousarray(
            np.broadcast_to(bout[c * VS:(c + 1) * VS], (B, VS)).astype(np.float32))
        in_maps.append(m)
    return in_maps


def _run(inputs, trace=False):
    nc = _build_program()
    in_maps = _prep_in_maps(inputs)
    res = run_bass_kernel_spmd(nc, in_maps, list(range(NCORES)), trace=trace)
    out = np.concatenate([res.results[c]["probs"] for c in range(NCORES)], axis=1)
    return out.astype(np.float32), res


def kernel(**inputs) -> np.ndarray:
    out, _ = _run(inputs, trace=False)
    return out



# revision 2
# speedup vs baseline: 1.2738x; 1.2738x over previous
"""Trainium2 Bass kernel for nn_Decoder (LSTM decoder + big output projection).

Model (VOCAB=32000, EM=256, UNITS=512, B=64, T=16):
  em     = emb_table[inputs]                      # [B,T,EM]
  xz     = em @ Wx + b                            # [B,T,4U] (precomputed input gates)
  scan:    z = xz_t + h @ Wh ; i,f,g,o = sigmoid(z)
           c = f*c + i*g ; h = o*sigmoid(c)       # 16 sequential steps
  logits = concat_t(h_t) @ Wout + bout            # [B, 8192] @ [8192, 32000]
  out    = softmax(logits)

Distribution over 8 NeuronCores:
  - The scan is replicated on every core (it is tiny and serial; replicating it
    avoids an AllGather of the hidden states).
  - Wout / softmax are column-sharded: core c owns vocab columns
    [c*4000, (c+1)*4000) and emits UNNORMALIZED exp(logits) for its shard.
    The softmax denominator (a [64] row-sum) and the bout column bias are
    applied on the host — exp(l + bout) = exp(l) * exp(bout) — so the device
    program has NO collective and the 8 cores run fully independently (no
    cross-core sync to absorb dispatch skew).
  - Weights are shipped to the device as bf16 (the 1 GB Wout read is the
    roofline floor; bf16 halves it).  All accumulation is fp32 in PSUM.

On-chip layout is "transposed": hidden state and gates live as [unit, batch]
tiles ([128 partitions, 64 batch]) so the recurrent matmul uses Wh as the
stationary operand and no per-step transposes are needed.  The per-step
output-projection partials (stationary = h_t^T, moving = streamed Wout rows)
run while the next step's gate math is on the Vector/Scalar engines, and the
Wout DMA stream overlaps everything.
"""

import numpy as np
import ml_dtypes
from contextlib import ExitStack

import concourse.bacc as bacc
import concourse.mybir as mybir
import concourse.tile as tile
from concourse.bass_utils import run_bass_kernel_spmd

VOCAB, EM, UNITS, B, T = 32000, 256, 512, 64, 16
NCORES = 8
VS = VOCAB // NCORES          # 4000 vocab columns per core
GU = 4 * UNITS                # 2048 gate units
NJ = GU // 128                # 16 gate m-tiles
KH = UNITS // 128             # 4 k-tiles of the hidden state
KE = EM // 128                # 2 k-tiles of the embedding
NTOK = B * T                  # 1024 tokens
NCH = 8                       # output-projection n-chunks per core
CH = VS // NCH                # 500 columns per chunk (<=512 PSUM bank limit)

BF16 = mybir.dt.bfloat16
F32 = mybir.dt.float32

_prog_cache = {}


def _build_program(_collective=True, _compile=True):
    """Trace + compile the single-core SPMD program (cached per process).

    (_collective is accepted for test-harness compatibility; the program no
    longer contains a collective in either mode.)"""
    key = ("nc", _compile)
    if key in _prog_cache:
        return _prog_cache[key]

    nc = bacc.Bacc("TRN2", target_bir_lowering=False, debug=False,
                   num_devices=NCORES)

    emt_d = nc.dram_tensor("emt", [128, KE, NTOK], BF16, kind="ExternalInput").ap()
    wx_d = nc.dram_tensor("wx", [128, KE, GU], BF16, kind="ExternalInput").ap()
    wh_d = nc.dram_tensor("wh", [128, KH, GU], BF16, kind="ExternalInput").ap()
    bt_d = nc.dram_tensor("bt", [128, NJ], F32, kind="ExternalInput").ap()
    h0_d = nc.dram_tensor("h0", [128, KH, B], BF16, kind="ExternalInput").ap()
    id_d = nc.dram_tensor("ident", [128, 128], BF16, kind="ExternalInput").ap()
    c0_d = nc.dram_tensor("c0", [128, KH * B], F32, kind="ExternalInput").ap()
    wout_d = nc.dram_tensor("wout", [T * UNITS, VS], BF16,
                            kind="ExternalInput").ap()
    out_d = nc.dram_tensor("eout", [B, VS], BF16, kind="ExternalOutput").ap()

    # [8192, VS] viewed as [128 partitions, 64 k-tiles, VS]
    wout_r = wout_d.rearrange("(s p) n -> p s n", p=128)

    with tile.TileContext(nc) as tc, ExitStack() as ctx:
        consts = ctx.enter_context(tc.tile_pool(name="consts", bufs=1))
        wout_pool = ctx.enter_context(tc.tile_pool(name="wout", bufs=6))
        psum_big = ctx.enter_context(tc.tile_pool(name="psb", bufs=2, space="PSUM"))
        psum_proj = ctx.enter_context(tc.tile_pool(name="psp", bufs=4, space="PSUM"))
        work = ctx.enter_context(tc.tile_pool(name="work", bufs=1))

        # ---- resident tensors (xz-phase inputs are DMA'd first so the PE can
        # start before the larger Wh / Wout loads land) -------------------------
        wx_sb = consts.tile([128, KE, GU], BF16, tag="wx")
        nc.sync.dma_start(out=wx_sb[:], in_=wx_d[:])
        emt = consts.tile([128, KE, NTOK], BF16, tag="emt")
        nc.sync.dma_start(out=emt[:], in_=emt_d[:])
        bt_sb = consts.tile([128, NJ], F32, tag="bt")
        nc.sync.dma_start(out=bt_sb[:], in_=bt_d[:])
        id_sb = consts.tile([128, 128], BF16, tag="ident")
        nc.sync.dma_start(out=id_sb[:], in_=id_d[:])
        wh_sb = consts.tile([128, KH, GU], BF16, tag="wh")
        nc.sync.dma_start(out=wh_sb[:], in_=wh_d[:])
        c_sb = consts.tile([128, KH * B], F32, tag="c")
        nc.sync.dma_start(out=c_sb[:], in_=c0_d[:])
        # hidden states for all steps (slot 0 = initial state), bf16 transposed
        hs_sb = consts.tile([128, T + 1, KH, B], BF16, tag="hs")
        nc.sync.dma_start(out=hs_sb[:, 0, :, :], in_=h0_d[:])
        # logits accumulator (written by the first projection step, no init DMA)
        logits = consts.tile([B, VS], F32, tag="logits")
        # xz = em @ Wx + b, transposed layout [gate-unit, (t, b)]
        xz_sb = consts.tile([128, NJ, T, B], BF16, tag="xz")

        # ---- xz = Wx^T @ em^T  (+ b folded in during PSUM evacuation) --------
        for j in range(NJ):
            ps = psum_big.tile([128, T * B], F32, tag="zps")
            for kt in range(KE):
                for nh in range(2):
                    nc.tensor.matmul(
                        ps[:, nh * 512:(nh + 1) * 512],
                        wx_sb[:, kt, j * 128:(j + 1) * 128],
                        emt[:, kt, nh * 512:(nh + 1) * 512],
                        start=(kt == 0), stop=(kt == KE - 1),
                    )
            nc.vector.tensor_scalar_add(
                xz_sb[:, j, :, :].rearrange("p t b -> p (t b)"),
                ps[:], bt_sb[:, j:j + 1])

        # ---- the scan + interleaved output projection ------------------------
        # Emission order matters for the scheduler: within step t we emit
        # z(t) matmuls FIRST, then the projection for step t-1, then the
        # gate math for t.  That way the PE chews on proj(t-1) while the
        # Vector/Scalar engines run gates(t) — without this the PE idles
        # ~5us per step waiting for h(t).
        def emit_proj(t, wts):
            # logits (+)= h_t @ Wout[512t:512(t+1), :]  (t==0 writes, else adds)
            for j in range(NCH):
                pp = psum_proj.tile([B, CH], F32, tag="pp")
                for kt in range(KH):
                    nc.tensor.matmul(
                        pp[:],
                        hs_sb[:, t + 1, kt, :],
                        wts[kt // 2][:, kt % 2, j * CH:(j + 1) * CH],
                        start=(kt == 0), stop=(kt == KH - 1),
                    )
                if t == 0:
                    nc.vector.tensor_copy(logits[:, j * CH:(j + 1) * CH], pp[:])
                else:
                    nc.vector.tensor_tensor(
                        logits[:, j * CH:(j + 1) * CH], pp[:],
                        logits[:, j * CH:(j + 1) * CH], mybir.AluOpType.add)

        prev_wts = None
        for t in range(T):
            # stream this step's 512 Wout rows (2 halves of 2 k-tiles each)
            wts = []
            for half in range(2):
                wt = wout_pool.tile([128, 2, VS], BF16, tag="wt")
                nc.sync.dma_start(
                    out=wt[:], in_=wout_r[:, 4 * t + 2 * half: 4 * t + 2 * half + 2, :])
                wts.append(wt)

            # z^T = xz_t + Wh^T @ h^T   [2048 units, 64 batch] in PSUM.
            # xz_t enters the accumulation group via an identity matmul so
            # no separate Vector-engine add sits on the serial chain.
            zt = psum_big.tile([128, NJ * B], F32, tag="zps")
            ztv = zt.rearrange("p (j b) -> p j b", b=B)
            for j in range(NJ):
                nc.tensor.matmul(ztv[:, j, :], id_sb[:],
                                 xz_sb[:, j, t, :], start=True, stop=False)
                for kt in range(KH):
                    nc.tensor.matmul(
                        ztv[:, j, :],
                        wh_sb[:, kt, j * 128:(j + 1) * 128],
                        hs_sb[:, t, kt, :],
                        start=False, stop=(kt == KH - 1),
                    )
            # all four gates use sigmoid; host permuted gate columns to
            # [i, g, f, o] so the activation can run in two halves and the
            # i*g product starts while f/o are still on the Scalar engine
            a_sb = work.tile([128, NJ, B], F32, tag="a")
            nc.scalar.activation(a_sb[:, 0:8, :], ztv[:, 0:8, :],
                                 mybir.ActivationFunctionType.Sigmoid)
            nc.scalar.activation(a_sb[:, 8:16, :], ztv[:, 8:16, :],
                                 mybir.ActivationFunctionType.Sigmoid)
            iT = a_sb[:, 0:4, :].rearrange("p j b -> p (j b)")
            gT = a_sb[:, 4:8, :].rearrange("p j b -> p (j b)")
            fT = a_sb[:, 8:12, :].rearrange("p j b -> p (j b)")
            oT = a_sb[:, 12:16, :].rearrange("p j b -> p (j b)")
            t1 = work.tile([128, KH * B], F32, tag="t1")
            t2 = work.tile([128, KH * B], F32, tag="t2")
            nc.vector.tensor_mul(t1[:], iT, gT)
            nc.vector.tensor_mul(t2[:], fT, c_sb[:])
            nc.vector.tensor_add(c_sb[:], t1[:], t2[:])
            sc = work.tile([128, KH * B], F32, tag="sc")
            nc.scalar.activation(sc[:], c_sb[:],
                                 mybir.ActivationFunctionType.Sigmoid)
            nc.vector.tensor_mul(
                hs_sb[:, t + 1, :, :].rearrange("p k b -> p (k b)"), oT, sc[:])
            if prev_wts is not None:
                emit_proj(t - 1, prev_wts)
            prev_wts = wts

        # ---- final projection step fused with exp + output streaming:
        # as soon as chunk j's last evacuation lands, exp it (bf16) and DMA it
        # out — the output stream overlaps the remaining chunks' matmuls.
        # the exp scratch reuses a Wout-pool slot (the stream is finished by
        # now) so it costs no extra SBUF
        exps = wout_pool.tile([B, VS], BF16, tag="wt")
        t = T - 1
        for j in range(NCH):
            pp = psum_proj.tile([B, CH], F32, tag="pp")
            for kt in range(KH):
                nc.tensor.matmul(
                    pp[:],
                    hs_sb[:, t + 1, kt, :],
                    prev_wts[kt // 2][:, kt % 2, j * CH:(j + 1) * CH],
                    start=(kt == 0), stop=(kt == KH - 1),
                )
            nc.vector.tensor_tensor(
                logits[:, j * CH:(j + 1) * CH], pp[:],
                logits[:, j * CH:(j + 1) * CH], mybir.AluOpType.add)
            nc.scalar.activation(
                exps[:, j * CH:(j + 1) * CH], logits[:, j * CH:(j + 1) * CH],
                mybir.ActivationFunctionType.Exp)
            nc.sync.dma_start(out=out_d[:, j * CH:(j + 1) * CH],
                              in_=exps[:, j * CH:(j + 1) * CH])

    if _compile:
        nc.compile()
    _prog_cache[key] = nc
    return nc


def _prep_in_maps(inputs):
    bf = ml_dtypes.bfloat16
    tok = np.asarray(inputs["inputs"]).astype(np.int64)        # [B, T]
    enc_h = np.asarray(inputs["enc_h"], np.float32)            # [B, U]
    enc_c = np.asarray(inputs["enc_c"], np.float32)            # [B, U]
    emb = np.asarray(inputs["emb_table"], np.float32)          # [V, EM]
    Wx = np.asarray(inputs["Wx"], np.float32)                  # [EM, 4U]
    Wh = np.asarray(inputs["Wh"], np.float32)                  # [U, 4U]
    b = np.asarray(inputs["b"], np.float32)                    # [4U]
    Wout = np.asarray(inputs["Wout"], np.float32)              # [T*U, V]

    # embedding lookup on host (pure data movement), shipped pre-transposed:
    # emt[p, k, i] = emb[tok_i, k*128 + p] with token order i = t*B + b
    em_flat = emb[tok.T.reshape(-1)]                           # [NTOK, EM]
    emt = em_flat.reshape(NTOK, KE, 128).transpose(2, 1, 0).astype(bf)

    # permute gate columns i,f,g,o -> i,g,f,o (lets the device split the
    # sigmoid into [i,g] / [f,o] halves)
    perm = np.r_[0:UNITS, 2 * UNITS:3 * UNITS, UNITS:2 * UNITS, 3 * UNITS:GU]
    Wx = Wx[:, perm]
    Wh = Wh[:, perm]
    b = b[perm]

    common = {
        "emt": np.ascontiguousarray(emt),
        "wx": np.ascontiguousarray(
            Wx.reshape(KE, 128, GU).transpose(1, 0, 2).astype(bf)),
        "wh": np.ascontiguousarray(
            Wh.reshape(KH, 128, GU).transpose(1, 0, 2).astype(bf)),
        "bt": np.ascontiguousarray(b.reshape(NJ, 128).T),
        "h0": np.ascontiguousarray(
            enc_h.T.reshape(KH, 128, B).transpose(1, 0, 2).astype(bf)),
        "ident": np.eye(128, dtype=bf),
        "c0": np.ascontiguousarray(
            enc_c.T.reshape(KH, 128, B).transpose(1, 0, 2).reshape(128, KH * B)),
    }
    wout_bf = Wout.astype(bf)
    in_maps = []
    for c in range(NCORES):
        m = dict(common)
        m["wout"] = np.ascontiguousarray(wout_bf[:, c * VS:(c + 1) * VS])
        in_maps.append(m)
    return in_maps


def _postprocess(eouts, bout):
    """Host-side softmax epilogue: probs = exp(l)*exp(bout) / row-sum."""
    e = np.concatenate([np.asarray(o, np.float32) for o in eouts], axis=1)
    bout = np.asarray(bout, np.float64)
    scale = np.exp(bout - bout.max()).astype(np.float32)
    e *= scale[None, :]
    e /= e.sum(axis=-1, keepdims=True)
    return e.astype(np.float32)


def _run(inputs, trace=False):
    nc = _build_program()
    in_maps = _prep_in_maps(inputs)
    res = run_bass_kernel_spmd(nc, in_maps, list(range(NCORES)), trace=trace)
    out = _postprocess([res.results[c]["eout"] for c in range(NCORES)],
                       inputs["bout"])
    return out, res


def kernel(**inputs) -> np.ndarray:
    out, _ = _run(inputs, trace=False)
    return out


# revision 5
# speedup vs baseline: 1.2741x; 1.0003x over previous
"""Trainium2 Bass kernel for nn_Decoder (LSTM decoder + big output projection).

Model (VOCAB=32000, EM=256, UNITS=512, B=64, T=16):
  em     = emb_table[inputs]                      # [B,T,EM]
  xz     = em @ Wx + b                            # [B,T,4U] (precomputed input gates)
  scan:    z = xz_t + h @ Wh ; i,f,g,o = sigmoid(z)
           c = f*c + i*g ; h = o*sigmoid(c)       # 16 sequential steps
  logits = concat_t(h_t) @ Wout + bout            # [B, 8192] @ [8192, 32000]
  out    = softmax(logits)

Distribution over 8 NeuronCores:
  - The scan is replicated on every core (it is tiny and serial; replicating it
    avoids an AllGather of the hidden states).
  - Wout / softmax are column-sharded: core c owns vocab columns
    [c*4000, (c+1)*4000) and emits UNNORMALIZED exp(logits) for its shard.
    The softmax denominator (a [64] row-sum) and the bout column bias are
    applied on the host — exp(l + bout) = exp(l) * exp(bout) — so the device
    program has NO collective and the 8 cores run fully independently (no
    cross-core sync to absorb dispatch skew).
  - Weights are shipped to the device as bf16 (the 1 GB Wout read is the
    roofline floor; bf16 halves it).  All accumulation is fp32 in PSUM.
  - All device inputs are packed into two blobs (one bf16, one f32) so each
    dispatch carries 3 buffer handles instead of 10 — per-call dispatch cost
    through the PJRT tunnel scales with the argument count.

On-chip layout is "transposed": hidden state and gates live as [unit, batch]
tiles ([128 partitions, 64 batch]) so the recurrent matmul uses Wh as the
stationary operand and no per-step transposes are needed.  The per-step
output-projection partials (stationary = h_t^T, moving = streamed Wout rows)
run while the next step's gate math is on the Vector/Scalar engines, and the
Wout DMA stream overlaps everything.  The last step's weights arrive as 8
column-chunk tiles (not 2 row-tiles) so the final projection+exp+store tail
pipelines with the end of the weight stream instead of serializing after it.
"""

import numpy as np
import ml_dtypes
from contextlib import ExitStack

import concourse.bacc as bacc
import concourse.mybir as mybir
import concourse.tile as tile
from concourse.bass_utils import run_bass_kernel_spmd

VOCAB, EM, UNITS, B, T = 32000, 256, 512, 64, 16
NCORES = 8
VS = VOCAB // NCORES          # 4000 vocab columns per core
GU = 4 * UNITS                # 2048 gate units
NJ = GU // 128                # 16 gate m-tiles
KH = UNITS // 128             # 4 k-tiles of the hidden state
KE = EM // 128                # 2 k-tiles of the embedding
NTOK = B * T                  # 1024 tokens
NCH = 8                       # output-projection n-chunks per core
CH = VS // NCH                # 500 columns per chunk (<=512 PSUM bank limit)

# bf16 blob column offsets (per-partition layout, [128, NB])
OFF_EMT = 0
OFF_WX = OFF_EMT + KE * NTOK          # 2048
OFF_WH = OFF_WX + KE * GU             # 6144
OFF_H0 = OFF_WH + KH * GU             # 14336
OFF_ID = OFF_H0 + KH * B              # 14592
OFF_WOUT = OFF_ID + 128               # 14720
NB = OFF_WOUT + (T * UNITS // 128) * VS   # 270720
# f32 blob column offsets ([128, NF])
OFF_BT = 0
OFF_C0 = OFF_BT + NJ                  # 16
NF = OFF_C0 + KH * B                  # 272

BF16 = mybir.dt.bfloat16
F32 = mybir.dt.float32

_prog_cache = {}


def _build_program(_collective=True, _compile=True):
    """Trace + compile the single-core SPMD program (cached per process).

    (_collective is accepted for test-harness compatibility; the program no
    longer contains a collective in either mode.)"""
    key = ("nc", _compile)
    if key in _prog_cache:
        return _prog_cache[key]

    nc = bacc.Bacc("TRN2", target_bir_lowering=False, debug=False,
                   num_devices=1, enable_partition_id=False)

    blob_d = nc.dram_tensor("blob", [128, NB], BF16, kind="ExternalInput").ap()
    fblob_d = nc.dram_tensor("fblob", [128, NF], F32, kind="ExternalInput").ap()
    out_d = nc.dram_tensor("eout", [B, VS], BF16, kind="ExternalOutput").ap()

    emt_d = blob_d[:, OFF_EMT:OFF_WX].rearrange("p (k t) -> p k t", k=KE)
    wx_d = blob_d[:, OFF_WX:OFF_WH].rearrange("p (k g) -> p k g", k=KE)
    wh_d = blob_d[:, OFF_WH:OFF_H0].rearrange("p (k g) -> p k g", k=KH)
    h0_d = blob_d[:, OFF_H0:OFF_ID].rearrange("p (k b) -> p k b", k=KH)
    id_d = blob_d[:, OFF_ID:OFF_WOUT]
    # Wout viewed as [128 partitions, 64 k-tiles, VS]
    wout_r = blob_d[:, OFF_WOUT:NB].rearrange("p (s n) -> p s n", n=VS)
    bt_d = fblob_d[:, OFF_BT:OFF_C0]
    c0_d = fblob_d[:, OFF_C0:NF]

    with tile.TileContext(nc) as tc, ExitStack() as ctx:
        consts = ctx.enter_context(tc.tile_pool(name="consts", bufs=1))
        wout_pool = ctx.enter_context(tc.tile_pool(name="wout", bufs=6))
        psum_big = ctx.enter_context(tc.tile_pool(name="psb", bufs=2, space="PSUM"))
        psum_proj = ctx.enter_context(tc.tile_pool(name="psp", bufs=4, space="PSUM"))
        work = ctx.enter_context(tc.tile_pool(name="work", bufs=1))

        def issue_step_tiles(t):
            # stream step t's 512 Wout rows as two [2-ktile x VS] row-tiles
            wts = []
            for half in range(2):
                wt = wout_pool.tile([128, 2, VS], BF16, tag="wt")
                nc.sync.dma_start(
                    out=wt[:],
                    in_=wout_r[:, 4 * t + 2 * half: 4 * t + 2 * half + 2, :])
                wts.append(wt)
            return wts

        # ---- resident tensors.  Order of DMA issue = queue order: the
        # xz-phase inputs first (PE can start), then step-0's Wout tiles (the
        # long stream starts ~1.5 MB in), then the scan constants (not needed
        # until the xz phase ends ~25 us later).
        wx_sb = consts.tile([128, KE, GU], BF16, tag="wx")
        nc.sync.dma_start(out=wx_sb[:], in_=wx_d[:])
        emt = consts.tile([128, KE, NTOK], BF16, tag="emt")
        nc.sync.dma_start(out=emt[:], in_=emt_d[:])
        bt_sb = consts.tile([128, NJ], F32, tag="bt")
        nc.sync.dma_start(out=bt_sb[:], in_=bt_d[:])
        id_sb = consts.tile([128, 128], BF16, tag="ident")
        nc.sync.dma_start(out=id_sb[:], in_=id_d[:])

        pending0 = issue_step_tiles(0)

        wh_sb = consts.tile([128, KH, GU], BF16, tag="wh")
        nc.sync.dma_start(out=wh_sb[:], in_=wh_d[:])
        c_sb = consts.tile([128, KH * B], F32, tag="c")
        nc.sync.dma_start(out=c_sb[:], in_=c0_d[:])
        # hidden states for all steps (slot 0 = initial state), bf16 transposed
        hs_sb = consts.tile([128, T + 1, KH, B], BF16, tag="hs")
        nc.sync.dma_start(out=hs_sb[:, 0, :, :], in_=h0_d[:])
        # logits accumulator (written by the first projection step, no init DMA)
        logits = consts.tile([B, VS], F32, tag="logits")
        # xz = em @ Wx + b, transposed layout [gate-unit, (t, b)]
        xz_sb = consts.tile([128, NJ, T, B], BF16, tag="xz")

        # ---- xz = Wx^T @ em^T  (+ b folded in during PSUM evacuation) --------
        for j in range(NJ):
            ps = psum_big.tile([128, T * B], F32, tag="zps")
            for kt in range(KE):
                for nh in range(2):
                    nc.tensor.matmul(
                        ps[:, nh * 512:(nh + 1) * 512],
                        wx_sb[:, kt, j * 128:(j + 1) * 128],
                        emt[:, kt, nh * 512:(nh + 1) * 512],
                        start=(kt == 0), stop=(kt == KE - 1),
                    )
            nc.vector.tensor_scalar_add(
                xz_sb[:, j, :, :].rearrange("p t b -> p (t b)"),
                ps[:], bt_sb[:, j:j + 1])

        # ---- the scan + interleaved output projection ------------------------
        # Emission order matters for the scheduler: within step t we emit
        # z(t) matmuls FIRST, then the projection for step t-1, then the
        # gate math for t.  That way the PE chews on proj(t-1) while the
        # Vector/Scalar engines run gates(t) — without this the PE idles
        # ~5us per step waiting for h(t).
        def emit_proj(t, wts):
            # logits (+)= h_t @ Wout[512t:512(t+1), :]  (t==0 writes, else adds)
            for j in range(NCH):
                pp = psum_proj.tile([B, CH], F32, tag="pp")
                for kt in range(KH):
                    nc.tensor.matmul(
                        pp[:],
                        hs_sb[:, t + 1, kt, :],
                        wts[kt // 2][:, kt % 2, j * CH:(j + 1) * CH],
                        start=(kt == 0), stop=(kt == KH - 1),
                    )
                if t == 0:
                    nc.vector.tensor_copy(logits[:, j * CH:(j + 1) * CH], pp[:])
                else:
                    nc.vector.tensor_tensor(
                        logits[:, j * CH:(j + 1) * CH], pp[:],
                        logits[:, j * CH:(j + 1) * CH], mybir.AluOpType.add)

        prev_wts = None
        for t in range(T):
            wts = pending0 if t == 0 else issue_step_tiles(t)

            # z^T = xz_t + Wh^T @ h^T   [2048 units, 64 batch] in PSUM.
            # xz_t enters the accumulation group via an identity matmul so
            # no separate Vector-engine add sits on the serial chain.
            zt = psum_big.tile([128, NJ * B], F32, tag="zps")
            ztv = zt.rearrange("p (j b) -> p j b", b=B)
            for j in range(NJ):
                nc.tensor.matmul(ztv[:, j, :], id_sb[:],
                                 xz_sb[:, j, t, :], start=True, stop=False)
                for kt in range(KH):
                    nc.tensor.matmul(
                        ztv[:, j, :],
                        wh_sb[:, kt, j * 128:(j + 1) * 128],
                        hs_sb[:, t, kt, :],
                        start=False, stop=(kt == KH - 1),
                    )
            # all four gates use sigmoid; host permuted gate columns to
            # [i, g, f, o] so the activation can run in two halves and the
            # i*g product starts while f/o are still on the Scalar engine
            a_sb = work.tile([128, NJ, B], F32, tag="a")
            nc.scalar.activation(a_sb[:, 0:8, :], ztv[:, 0:8, :],
                                 mybir.ActivationFunctionType.Sigmoid)
            nc.scalar.activation(a_sb[:, 8:16, :], ztv[:, 8:16, :],
                                 mybir.ActivationFunctionType.Sigmoid)
            iT = a_sb[:, 0:4, :].rearrange("p j b -> p (j b)")
            gT = a_sb[:, 4:8, :].rearrange("p j b -> p (j b)")
            fT = a_sb[:, 8:12, :].rearrange("p j b -> p (j b)")
            oT = a_sb[:, 12:16, :].rearrange("p j b -> p (j b)")
            t1 = work.tile([128, KH * B], F32, tag="t1")
            t2 = work.tile([128, KH * B], F32, tag="t2")
            nc.vector.tensor_mul(t1[:], iT, gT)
            nc.vector.tensor_mul(t2[:], fT, c_sb[:])
            nc.vector.tensor_add(c_sb[:], t1[:], t2[:])
            sc = work.tile([128, KH * B], F32, tag="sc")
            nc.scalar.activation(sc[:], c_sb[:],
                                 mybir.ActivationFunctionType.Sigmoid)
            nc.vector.tensor_mul(
                hs_sb[:, t + 1, :, :].rearrange("p k b -> p (k b)"), oT, sc[:])
            if prev_wts is not None:
                emit_proj(t - 1, prev_wts)
            prev_wts = wts

        # ---- final projection step fused with exp + output streaming.
        # Two PSUM waves: wave A (chunks 0..3) issues its kt0/kt1 matmuls
        # first — they only need the first half-tile, which lands one DMA
        # slot before the last — so the PE works through them while the
        # final 2 MB of the weight stream is still in flight.  Each chunk's
        # evacuation chains into exp (bf16) and its 64 KB output DMA.
        exps = wout_pool.tile([B, VS], BF16, tag="wt")
        t = T - 1
        HALF = NCH // 2
        ppA = []
        for j in range(HALF):
            pp = psum_proj.tile([B, CH], F32, tag="pp")
            for kt in range(2):
                nc.tensor.matmul(
                    pp[:], hs_sb[:, t + 1, kt, :],
                    prev_wts[0][:, kt, j * CH:(j + 1) * CH],
                    start=(kt == 0), stop=False)
            ppA.append(pp)

        def finish_chunk(j, pp, kts):
            for kt in kts:
                nc.tensor.matmul(
                    pp[:], hs_sb[:, t + 1, kt, :],
                    prev_wts[kt // 2][:, kt % 2, j * CH:(j + 1) * CH],
                    start=(kt == 0), stop=(kt == KH - 1))
            nc.vector.tensor_tensor(
                logits[:, j * CH:(j + 1) * CH], pp[:],
                logits[:, j * CH:(j + 1) * CH], mybir.AluOpType.add)
            nc.scalar.activation(
                exps[:, j * CH:(j + 1) * CH], logits[:, j * CH:(j + 1) * CH],
                mybir.ActivationFunctionType.Exp)
            nc.sync.dma_start(out=out_d[:, j * CH:(j + 1) * CH],
                              in_=exps[:, j * CH:(j + 1) * CH])

        for j in range(HALF):
            finish_chunk(j, ppA[j], (2, 3))
        for j in range(HALF, NCH):
            pp = psum_proj.tile([B, CH], F32, tag="pp")
            finish_chunk(j, pp, (0, 1, 2, 3))

    if _compile:
        nc.compile()
    _prog_cache[key] = nc
    return nc


def _prep_in_maps(inputs):
    bf = ml_dtypes.bfloat16
    tok = np.asarray(inputs["inputs"]).astype(np.int64)        # [B, T]
    enc_h = np.asarray(inputs["enc_h"], np.float32)            # [B, U]
    enc_c = np.asarray(inputs["enc_c"], np.float32)            # [B, U]
    emb = np.asarray(inputs["emb_table"], np.float32)          # [V, EM]
    Wx = np.asarray(inputs["Wx"], np.float32)                  # [EM, 4U]
    Wh = np.asarray(inputs["Wh"], np.float32)                  # [U, 4U]
    b = np.asarray(inputs["b"], np.float32)                    # [4U]
    Wout = np.asarray(inputs["Wout"], np.float32)              # [T*U, V]

    # embedding lookup on host (pure data movement), shipped pre-transposed:
    # emt[p, k, i] = emb[tok_i, k*128 + p] with token order i = t*B + b
    em_flat = emb[tok.T.reshape(-1)]                           # [NTOK, EM]
    emt = em_flat.reshape(NTOK, KE, 128).transpose(2, 1, 0).astype(bf)

    # permute gate columns i,f,g,o -> i,g,f,o (lets the device split the
    # sigmoid into [i,g] / [f,o] halves)
    perm = np.r_[0:UNITS, 2 * UNITS:3 * UNITS, UNITS:2 * UNITS, 3 * UNITS:GU]
    Wx = Wx[:, perm]
    Wh = Wh[:, perm]
    b = b[perm]

    head = np.concatenate([
        emt.reshape(128, -1),
        Wx.reshape(KE, 128, GU).transpose(1, 0, 2).astype(bf).reshape(128, -1),
        Wh.reshape(KH, 128, GU).transpose(1, 0, 2).astype(bf).reshape(128, -1),
        enc_h.T.reshape(KH, 128, B).transpose(1, 0, 2).astype(bf).reshape(128, -1),
        np.eye(128, dtype=bf),
    ], axis=1)                                                 # [128, OFF_WOUT]
    fblob = np.concatenate([
        b.reshape(NJ, 128).T,
        enc_c.T.reshape(KH, 128, B).transpose(1, 0, 2).reshape(128, KH * B),
    ], axis=1).astype(np.float32)                              # [128, NF]
    fblob = np.ascontiguousarray(fblob)

    # Wout as [128 partitions, 64 k-tiles, 8 cores, VS] in bf16
    wview = Wout.astype(bf).reshape(T * UNITS // 128, 128, NCORES, VS)
    in_maps = []
    for c in range(NCORES):
        wc = wview[:, :, c, :].transpose(1, 0, 2).reshape(128, -1)
        blob = np.ascontiguousarray(np.concatenate([head, wc], axis=1))
        in_maps.append({"blob": blob, "fblob": fblob})
    return in_maps


def _postprocess(eouts, bout):
    """Host-side softmax epilogue: probs = exp(l)*exp(bout) / row-sum."""
    e = np.concatenate([np.asarray(o, np.float32) for o in eouts], axis=1)
    bout = np.asarray(bout, np.float64)
    scale = np.exp(bout - bout.max()).astype(np.float32)
    e *= scale[None, :]
    e /= e.sum(axis=-1, keepdims=True)
    return e.astype(np.float32)


def _run(inputs, trace=False):
    nc = _build_program()
    in_maps = _prep_in_maps(inputs)
    res = run_bass_kernel_spmd(nc, in_maps, list(range(NCORES)), trace=trace)
    out = _postprocess([res.results[c]["eout"] for c in range(NCORES)],
                       inputs["bout"])
    return out, res


def kernel(**inputs) -> np.ndarray:
    out, _ = _run(inputs, trace=False)
    return out
